# revision 68
# baseline (speedup 1.0000x reference)
"""Trainium2 Bass kernel for the BiDAF-style AttentionFlow layer.

Reference computation (per batch element b):
    s0 = c @ proj_c                      # [Lc, 1]
    s1 = (q @ proj_q)^T                  # [1, Lq]
    s2 = (c * proj_cq) @ q^T             # [Lc, Lq]
    sim = s0 + s1 + s2
    a_c2q = softmax(sim, axis=-1);  c2q = a_c2q @ q
    a_q2c = softmax(max(sim, -1));  q2c = a_q2c @ c        (broadcast over Lc)
    out = concat(c, c2q, c*q2c, c*c2q, axis=-1)

Shapes: B=32, Lc=512, Lq=64, D=1024.  Data-parallel over batch: 8 NeuronCores,
4 batch elements each.  No collectives.

Dispatch cost in this environment is dominated by per-operand-buffer fixed
overhead (~1.4 ms each) and input-transfer bytes through the axon tunnel,
so the device I/O is aggressively packed:
  * ONE input buffer per core: [ c int8 | q f16 | cscale f16 | projs f16 ].
    c is per-row symmetric-int8 quantized on the host (2 MiB instead of
    4 MiB) and dequantized to f16 on-device (spread over gpsimd, scalar
    and vector); q stays f16 because it feeds the c2q output directly.
  * ONE output buffer per core: [bpc, Lc+1, D] f16 — rows 0..Lc-1 hold
    c2q, row Lc carries the unnormalized q2c attention weights w.  The
    c, q2c, c*q2c and c*c2q output sections are assembled on the host
    from the original f32 c: the products are pure redundancy over the
    wire, and q2c itself is one tiny [Lc] x [Lc, D] matvec per batch,
    so shipping w instead deletes the whole device q2c path (48 PE
    matmuls, 8 scalar copies per core) and frees two PSUM banks for
    deeper S2T/transpose double-buffering.

On-chip structure (per batch element):
  * S2T = s2^T [Lq, Lc] via PE matmuls (contraction dim D on partitions for
    both operands, so C is transposed on the PE; proj_cq is folded into the
    transposed-q copy).  A 65th stationary column equal to proj_c makes row
    64 of the same matmul compute s0 — no separate s0 matmuls.
  * s1 = rowsum(q * proj_q-broadcast) via one tensor_tensor_reduce whose
    accumulator starts at the softmax shift, giving the E-bias column
    directly (no q^T copy, no s1 matmuls).
  * E = exp(S2T + s1 - 8) rows 0..63, E[64] = exp(s0) (bias 0 on row 64).
    The -8 shift keeps E inside fp16 range; it cancels in both softmaxes.
  * colsum/colmax of E via 4 PE transposes into one packed PSUM tile, then
    ONE strided reduce_sum + ONE reduce_max over all 4 chunks.
  * a_q2c weights w = E[64] * colmax(E), shipped out via one rearranged
    1 KB DMA; normalization and the q2c matvec happen host-side.
  * The unnormalized E serves directly as the matmul lhsT for c2q = E^T @ q;
    normalization by 1/colsum happens in the PSUM->SBUF copy.
  * Software pipelining: per-batch work is split into stage A (transposes,
    S2T, E, reductions, w store) and stage B (c2q, stores), interleaved
    with input DMAs running two batches ahead, so the in-order engine
    queues never head-block a later batch's independent work behind an
    earlier batch's cross-engine chain.
"""

import sys

sys.path.insert(0, "/opt/trn_rl_repo")

import numpy as np

import concourse.bacc as bacc
import concourse.mybir as mybir
import concourse.tile as tile
from concourse import masks

F32 = mybir.dt.float32
F16 = mybir.dt.float16
AF = mybir.ActivationFunctionType
AX = mybir.AxisListType
ALU = mybir.AluOpType

N_CORES = 8
B, LC, LQ, D = 32, 512, 64, 1024
BPC = B // N_CORES          # batch elements per core (4)
NCC = LC // 128             # c-chunks (4)
NDC = D // 128              # d-chunks (8)
LQ1 = LQ + 1                # 65: row 64 of E carries exp(s0)
DOUT = 4 * D                # full output width (host side)
SHIFT = -8.0                # softmax shift; keeps E in fp16 range

# single packed input blob, one ExternalInput instead of five — each extra
# operand buffer costs ~1.4 ms of fixed per-dispatch overhead through the
# axon tunnel.  c ships as int8 with a per-row f16 scale (dequantized
# on-device, split across vector+gpsimd); q and the tiny proj vectors ship
# as f16 (q's precision feeds the c2q output directly and halves the logit
# noise, for only +0.25 MiB/core).  Layout (bytes):
#   [ c int8 | q f16 | cscale f16 | proj_c | proj_q | proj_cq ]
I8 = mybir.dt.int8
CB = BPC * LC * D                  # c bytes (int8)
QB = BPC * LQ * D * 2              # q bytes (f16)
CS_OFF = CB + QB                   # cscale offset
S1_OFF = CS_OFF + BPC * LC * 2     # host-computed s1 bias column (f32)
PC_OFF = S1_OFF + BPC * LQ1 * 4    # proj_c offset
PQ_OFF = PC_OFF + 2 * D
PCQ_OFF = PQ_OFF + 2 * D
BLOB_BYTES = PCQ_OFF + 2 * D


def build_bass(bpc=BPC):
    nc = bacc.Bacc()
    blob = nc.declare_dram_parameter("blob", [BLOB_BYTES], I8, isOutput=False)
    c_ext = blob[0:CB].rearrange("(b l d) -> b l d", b=bpc, l=LC)
    q_ext = blob[CB : CB + QB].bitcast(F16).rearrange("(b l d) -> b l d", b=bpc, l=LQ)
    csc_ext = blob[CS_OFF:S1_OFF].bitcast(F16).rearrange("(b l) -> b l", b=bpc)
    s1_ext = blob[S1_OFF:PC_OFF].bitcast(F32).rearrange("(b l) -> b l", b=bpc)
    pc_ext = blob[PC_OFF:PQ_OFF].bitcast(F16)
    pcq_ext = blob[PCQ_OFF:BLOB_BYTES].bitcast(F16)
    # rows 0..LC-1: c2q; row LC: the unnormalized q2c attention weights
    # (first LC cols).  The c, q2c, c*q2c, c*c2q output sections are formed
    # on the host from the original f32 c and these tensors.
    out_ext = nc.declare_dram_parameter("out", [bpc, LC + 1, D], F16, isOutput=True)

    with tile.TileContext(nc) as tc:
        _build(nc, tc, c_ext, q_ext, csc_ext, s1_ext, pc_ext, pcq_ext,
               out_ext, bpc)
    nc.finalize()
    return nc


def _build(nc, tc, c_ext, q_ext, csc_ext, s1_ext, pc_ext, pcq_ext,
           out_ext, bpc):
    from contextlib import ExitStack

    with ExitStack() as ctx:
        const = ctx.enter_context(tc.tile_pool(name="const", bufs=1))
        cpool = ctx.enter_context(tc.tile_pool(name="cpool", bufs=4))
        c8pool = ctx.enter_context(tc.tile_pool(name="c8pool", bufs=2))
        qpool = ctx.enter_context(tc.tile_pool(name="qpool", bufs=4))
        ctTp = ctx.enter_context(tc.tile_pool(name="ctT", bufs=2))
        epool = ctx.enter_context(tc.tile_pool(name="epool", bufs=3))
        small = ctx.enter_context(tc.tile_pool(name="small", bufs=3))
        bpools = ctx.enter_context(tc.tile_pool(name="bpool", bufs=2))
        outp = ctx.enter_context(tc.tile_pool(name="outp", bufs=8))
        ps_t = ctx.enter_context(tc.tile_pool(name="ps_t", bufs=3, space="PSUM"))
        ps_s2 = ctx.enter_context(tc.tile_pool(name="ps_s2", bufs=2, space="PSUM"))
        ps_cq = ctx.enter_context(tc.tile_pool(name="ps_cq", bufs=2, space="PSUM"))
        ps_sm = ctx.enter_context(tc.tile_pool(name="ps_sm", bufs=1, space="PSUM"))

        # ---- constants ----
        ident = const.tile([128, 128], F16)
        masks.make_identity(nc, ident[:])


        # ---- per-batch state + loads (batch 0's DMAs issue before the
        # const DMAs so compute can start as early as possible) ----
        st = [dict() for _ in range(bpc)]

        def load(b):
            # q first (small, unblocks the q-side ops); c split by d-halves
            # so the dequant of chunk group g=0 can start after half 0.
            # c dequant spreads across gpsimd/scalar/vector so no single
            # engine serializes all four batches' conversions.
            q16 = qpool.tile([LQ, D], F16, tag="q16")
            nc.sync.dma_start(q16[:], q_ext[b])
            s1x = small.tile([LQ1, 1], F32, tag="s1x")
            nc.sync.dma_start(s1x[:], s1_ext[b].rearrange("(l o) -> l o", o=1))
            st[b]["s1x"] = s1x

            c8t = c8pool.tile([128, NCC, D], I8, tag="c8")
            cs16 = small.tile([128, NCC], F16, tag="cs16")
            nc.sync.dma_start(cs16[:], csc_ext[b].rearrange("(i p) -> p i", p=128))
            cs32 = small.tile([128, NCC], F32, tag="cs32")
            nc.vector.tensor_copy(cs32[:], cs16[:])
            for g in range(2):
                nc.sync.dma_start(
                    c8t[:, :, g * 512 : (g + 1) * 512],
                    c_ext[b, :, g * 512 : (g + 1) * 512].rearrange(
                        "(i p) d -> p i d", p=128
                    ),
                )
            c16 = cpool.tile([128, NCC, D], F16, tag="c16")
            for g in range(2):
                for i in range(NCC):
                    sl = slice(g * 512, (g + 1) * 512)
                    k = g * NCC + i
                    # dequant leans on gpsimd + scalar; DVE is the
                    # saturated mid-kernel engine (trace-verified), so it
                    # only takes a quarter of the conversions
                    if k % 4 in (0, 2):
                        nc.gpsimd.tensor_scalar_mul(
                            c16[:, i, sl], in0=c8t[:, i, sl],
                            scalar1=cs32[:, i : i + 1],
                        )
                    elif k % 4 == 3 and b % 2 == 0:
                        nc.scalar.activation(
                            c16[:, i, sl], c8t[:, i, sl], AF.Copy,
                            bias=0.0, scale=cs32[:, i : i + 1],
                        )
                    elif k % 4 == 3:
                        nc.gpsimd.tensor_scalar_mul(
                            c16[:, i, sl], in0=c8t[:, i, sl],
                            scalar1=cs32[:, i : i + 1],
                        )
                    else:
                        nc.vector.tensor_scalar_mul(
                            c16[:, i, sl], in0=c8t[:, i, sl],
                            scalar1=cs32[:, i : i + 1],
                        )
            st[b]["c16"], st[b]["q16"] = c16, q16

        load(0)

        # proj vectors as [128, NDC]: partition = d % 128, column = d // 128
        # (f16 direct from the blob — the compute below already ran on f16
        # copies of the f32 originals, so nothing is lost)
        wcq16 = const.tile([128, NDC], F16)
        nc.sync.dma_start(wcq16[:], pcq_ext.rearrange("(j p) -> p j", p=128))
        wcq = const.tile([128, NDC], F32)
        nc.vector.tensor_copy(wcq[:], wcq16[:])
        wc16 = const.tile([128, NDC], F16)
        nc.sync.dma_start(wc16[:], pc_ext.rearrange("(j p) -> p j", p=128))

        def stage_a(b):
            c16, q16 = st[b]["c16"], st[b]["q16"]

            # transpose q (d on partitions); fold proj_cq; col 64 = proj_c
            qwT = qpool.tile([128, NDC, LQ1], F16, tag="qwT")
            for j in range(NDC):
                tp = ps_t.tile([128, 512], F16, tag="tp")
                nc.tensor.transpose(
                    tp[:, :LQ], q16[:, j * 128 : (j + 1) * 128], ident[:LQ, :LQ]
                )
                nc.vector.tensor_scalar_mul(
                    qwT[:, j, :LQ], in0=tp[:, :LQ], scalar1=wcq[:, j : j + 1]
                )
                nc.gpsimd.tensor_copy(qwT[:, j, LQ:LQ1], wc16[:, j : j + 1])

            # transpose C (fp16), d on partitions
            ctT = ctTp.tile([128, NDC, 512], F16, tag="ctT")
            for j in range(NDC):
                tpc = ps_t.tile([128, 512], F16, tag="tp")
                for i in range(NCC):
                    nc.tensor.transpose(
                        tpc[:, i * 128 : (i + 1) * 128],
                        c16[:, i, j * 128 : (j + 1) * 128],
                        ident[:],
                    )
                if j % 2 == 1:
                    nc.scalar.copy(ctT[:, j, :], tpc[:])
                else:
                    nc.vector.tensor_copy(ctT[:, j, :], tpc[:])

            # S2T [65, Lc]: rows 0..63 = s2^T, row 64 = s0
            s2ps = ps_s2.tile([LQ1, LC], F32, tag="s2")
            for j in range(NDC):
                nc.tensor.matmul(
                    s2ps[:],
                    qwT[:, j, :],
                    ctT[:, j, :],
                    start=(j == 0),
                    stop=(j == NDC - 1),
                )

            # s1 + shift arrives host-computed as a ready bias column
            s1x = st[b]["s1x"]

            # E = exp(S2T + bias) [65, Lc] fp16
            Et = epool.tile([LQ1, LC], F16, tag="E")
            nc.scalar.activation(Et[:], s2ps[:], AF.Exp, bias=s1x[:], scale=1.0)

            # packed E^T [128, NCC, 66]; stride padded to 66 so each
            # chunk's PSUM byte offset stays 4-byte aligned (66*2 = 132)
            etp = ps_sm.tile([128, NCC, LQ1 + 1], F16, tag="etp")
            for i in range(NCC):
                nc.tensor.transpose(
                    etp[:, i, :LQ1], Et[:, i * 128 : (i + 1) * 128], ident[:LQ1, :LQ1]
                )
            emax = small.tile([128, NCC], F16, tag="emax")
            nc.vector.reduce_max(emax[:], etp[:, :, :LQ], axis=AX.X)
            den = small.tile([128, NCC], F32, tag="den")
            nc.vector.reduce_sum(den[:], etp[:, :, :LQ], axis=AX.X)
            w = small.tile([128, NCC], F32, tag="w")
            nc.vector.tensor_mul(w[:], etp[:, :, LQ:LQ1], emax[:])
            w16 = small.tile([128, NCC], F16, tag="w16")
            nc.vector.tensor_copy(w16[:], w[:])
            rden = small.tile([128, NCC], F32, tag="rden")
            nc.vector.reciprocal(rden[:], den[:])
            # ship the unnormalized q2c weights themselves in the spare
            # output row (dram-side rearrange scatters [128, NCC] to the
            # l = i*128+p order); the host does the tiny (w/sum(w)) @ c
            # matvec against the exact f32 c it already holds.  This
            # deletes the whole device q2c path (48 PE matmuls, 8 scalar
            # copies per core) and frees two PSUM banks.
            nc.sync.dma_start(
                out_ext[b, LC : LC + 1, 0:LC].rearrange("o (i p) -> (o p) i", p=128),
                w16[:],
            )
            st[b]["Et"], st[b]["w16"] = Et, w16
            st[b]["rden"] = rden

        def stage_b2(b):
            # per c-chunk: c2q; each chunk streams out as soon as its
            # normalization copy lands
            q16 = st[b]["q16"]
            Et, rden = st[b]["Et"], st[b]["rden"]
            for i in range(NCC):
                ost = outp.tile([128, D], F16, tag="ost")
                for h in range(2):
                    cq = ps_cq.tile([128, 512], F32, tag="cq")
                    nc.tensor.matmul(
                        cq[:],
                        Et[:LQ, i * 128 : (i + 1) * 128],
                        q16[:, h * 512 : (h + 1) * 512],
                        start=True,
                        stop=True,
                    )
                    # normalization copies split scalar/DVE so neither
                    # serializes all 32 of them per core (gpsimd is out:
                    # it has no PSUM port); scalar takes one extra per
                    # batch to equalize the two queues
                    if h == 0 or i == 3:
                        nc.scalar.activation(
                            ost[:, h * 512 : (h + 1) * 512],
                            cq[:],
                            AF.Copy,
                            bias=0.0,
                            scale=rden[:, i : i + 1],
                        )
                    else:
                        nc.vector.tensor_scalar_mul(
                            ost[:, h * 512 : (h + 1) * 512],
                            in0=cq[:],
                            scalar1=rden[:, i : i + 1],
                        )
                r0, r1 = i * 128, (i + 1) * 128
                nc.sync.dma_start(out_ext[b, r0:r1, :], ost[:])

        # ---- software-pipelined schedule ----
        # All loads issue upfront (SBUF holds every batch), then stages
        # interleave so the in-order engine queues never head-block a later
        # batch's independent work behind an earlier batch's cross-engine
        # chain: A = transposes/S2T/E/reductions, B2a = c2q + store (fast
        # path), B1 = q2c weight chain, B2b = products + store (slow path).
        # loads run two batches ahead instead of all-upfront: issuing every
        # batch's dequant first head-blocks the in-order DVE/gpsimd queues
        # and leaves PE idle for the first ~10us (trace-verified)
        if bpc > 1:
            load(1)
        stage_a(0)
        if bpc > 2:
            load(2)
        for b in range(bpc):
            if b + 1 < bpc:
                stage_a(b + 1)
                if b + 3 < bpc:
                    load(b + 3)
            stage_b2(b)


_NC_CACHE = None


def _get_nc():
    global _NC_CACHE
    if _NC_CACHE is None:
        _NC_CACHE = build_bass()
    return _NC_CACHE


def build_runner(nc, n_cores=N_CORES):
    """Jitted SPMD dispatcher for nc with the minimal operand set.

    The stock run_bass_kernel_spmd path binds a zero-filled buffer for every
    ExternalOutput as an extra operand (donated, so partially-written outputs
    see zeros).  This kernel writes every element of its single output, so
    those operands are dead weight — the NEFF rename maps the output tensor
    to the custom-call result slot and the zero operand binds to nothing.
    Dropping them (and donation) removes out-sized host->device traffic per
    dispatch.  Returns (f, in_names, out_names, out_avals); call as
    f(*concatenated_inputs) -> per-core-stacked outputs.
    """
    import jax
    from concourse import bass2jax
    from concourse.bass2jax import _bass_exec_p, partition_id_tensor
    from jax.sharding import Mesh, PartitionSpec
    from jax.experimental.shard_map import shard_map

    bass2jax.install_neuronx_cc_hook()

    partition_name = nc.partition_id_tensor.name if nc.partition_id_tensor else None
    in_names, out_names, out_avals = [], [], []
    for alloc in nc.m.functions[0].allocations:
        if not isinstance(alloc, mybir.MemoryLocationSet):
            continue
        name = alloc.memorylocations[0].name
        if alloc.kind == "ExternalInput":
            if name != partition_name:
                in_names.append(name)
        elif alloc.kind == "ExternalOutput":
            out_names.append(name)
            out_avals.append(
                jax.core.ShapedArray(tuple(alloc.tensor_shape), mybir.dt.np(alloc.dtype))
            )
    all_in_names = list(in_names)
    if partition_name is not None:
        all_in_names.append(partition_name)

    def _body(*args):
        operands = list(args)
        if partition_name is not None:
            operands.append(partition_id_tensor())
        outs = _bass_exec_p.bind(
            *operands,
            out_avals=tuple(out_avals),
            in_names=tuple(all_in_names),
            out_names=tuple(out_names),
            lowering_input_output_aliases=(),
            sim_require_finite=True,
            sim_require_nnan=True,
            nc=nc,
        )
        return tuple(outs)

    devices = jax.devices()[:n_cores]
    mesh = Mesh(np.asarray(devices), ("core",))
    f = jax.jit(
        shard_map(
            _body,
            mesh=mesh,
            in_specs=(PartitionSpec("core"),) * len(in_names),
            out_specs=(PartitionSpec("core"),) * len(out_names),
            check_rep=False,
        ),
        keep_unused=True,
    )
    return f, in_names, out_names, out_avals


_RUNNER_CACHE = None


def _get_runner():
    global _RUNNER_CACHE
    if _RUNNER_CACHE is None:
        _RUNNER_CACHE = build_runner(_get_nc())
    return _RUNNER_CACHE


def _quant_rows(a):
    """Per-row symmetric int8 quantization; scale stored as f16."""
    flat = a.reshape(-1, a.shape[-1]).astype(np.float32)
    s16 = np.maximum(np.abs(flat).max(axis=-1) / 127.0, 1e-6).astype(np.float16)
    q = np.clip(
        np.rint(flat / s16.astype(np.float32)[:, None]), -127, 127
    ).astype(np.int8)
    return q.reshape(a.shape), s16.reshape(a.shape[:-1])


def make_in_maps(c, q, proj_c, proj_q, proj_cq):
    """Shard + quantize full f32 inputs into per-core packed blobs."""
    q32 = np.ascontiguousarray(q, dtype=np.float32)
    c8, cs16 = _quant_rows(np.ascontiguousarray(c, dtype=np.float32))
    q16 = q32.astype(np.float16)
    pc16 = np.asarray(proj_c, dtype=np.float32).astype(np.float16).ravel()
    pcq16 = np.asarray(proj_cq, dtype=np.float32).astype(np.float16).ravel()
    # the tiny s1 = q @ proj_q bias ships pre-computed (and pre-shifted);
    # row LQ is the zero bias for the exp(s0) row of E
    s1 = np.zeros((B, LQ1), np.float32)
    s1[:, :LQ] = (q32 @ np.asarray(proj_q, dtype=np.float32))[..., 0] + SHIFT
    in_maps = []
    for r in range(N_CORES):
        sl = slice(r * BPC, (r + 1) * BPC)
        blob = np.empty(BLOB_BYTES, np.int8)
        blob[0:CB] = c8[sl].ravel()
        blob[CB : CB + QB] = q16[sl].ravel().view(np.int8)
        blob[CS_OFF:S1_OFF] = cs16[sl].ravel().view(np.int8)
        blob[S1_OFF:PC_OFF] = s1[sl].ravel().view(np.int8)
        blob[PC_OFF:PQ_OFF] = pc16.view(np.int8)
        blob[PCQ_OFF:BLOB_BYTES] = pcq16.view(np.int8)
        in_maps.append({"blob": blob})
    return in_maps


def assemble_out(c, dev_outs):
    """Full f32 output from the original f32 c and per-core device results.

    The device returns c2q (rows 0..LC-1) and the broadcast q2c row (row LC)
    per batch element; the c, c*q2c and c*c2q sections are assembled here
    from the original full-precision c.
    """
    out = np.empty((B, LC, DOUT), np.float32)
    out[..., :D] = c
    for r in range(N_CORES):
        sl = slice(r * BPC, (r + 1) * BPC)
        dev = dev_outs[r].astype(np.float32)
        c2q = dev[:, :LC, :]
        # row LC carries the unnormalized q2c attention weights; the
        # normalization and the tiny [Lc] x [Lc, D] matvec happen here
        # against the exact f32 c
        wq = dev[:, LC, :LC]
        a = wq / wq.sum(axis=-1, keepdims=True)
        q2c = np.einsum("bl,bld->bd", a, c[sl])[:, None, :]
        out[sl, :, D : 2 * D] = c2q
        np.multiply(c[sl], q2c, out=out[sl, :, 2 * D : 3 * D])
        np.multiply(c[sl], c2q, out=out[sl, :, 3 * D : 4 * D])
    return out


def kernel(c, q, proj_c, proj_q, proj_cq):
    c = np.ascontiguousarray(c, dtype=np.float32)
    import jax

    in_maps = make_in_maps(c, q, proj_c, proj_q, proj_cq)
    f, in_names, out_names, out_avals = _get_runner()
    args = [
        jax.device_put(
            np.concatenate([np.asarray(in_maps[r][k]) for r in range(N_CORES)], axis=0)
        )
        for k in in_names
    ]
    oi = out_names.index("out")
    for attempt in range(3):
        outs = f(*args)
        dev = np.asarray(outs[oi]).reshape(N_CORES, *out_avals[oi].shape)
        # the first execution after a fresh NEFF load occasionally returns
        # corrupted data on this shared tunnel (observed once in ~50 runs);
        # the computed regions (c2q rows + the w prefix of row LC) are
        # NaN-free by construction, so retry on NaN there
        if not (
            np.isnan(dev[:, :, :LC, :]).any()
            or np.isnan(dev[:, :, LC, :LC]).any()
        ):
            break
    return assemble_out(c, [dev[r] for r in range(N_CORES)])


if __name__ == "__main__":
    rng = np.random.default_rng(0)
    c = rng.standard_normal((B, LC, D)).astype(np.float32)
    q = rng.standard_normal((B, LQ, D)).astype(np.float32)
    pc = (rng.standard_normal((D, 1)) * 0.04).astype(np.float32)
    pq = (rng.standard_normal((D, 1)) * 0.04).astype(np.float32)
    pcq = (rng.standard_normal((1, 1, D)) * 0.04).astype(np.float32)
    out = kernel(c=c, q=q, proj_c=pc, proj_q=pq, proj_cq=pcq)
    print("out", out.shape, out.dtype, float(np.abs(out).max()))

